# revision 43
# baseline (speedup 1.0000x reference)
"""BinaryTreeGRU Trainium2 kernel (bit-reversed chunk-major layout).

Batch of B=64 complete binary trees (L=512 leaves, 1023 nodes each),
data-parallel over trees across 8 NeuronCores (8 trees/core).

Every level's node-columns are stored in BIT-REVERSED order of the
natural per-core column index: the children of br-position k at level lv
live at br-positions k (left) and n_prev/2 + k (right) of level lv-1,
so child reads are contiguous halves and h writes are contiguous — no
strided element-wise ops anywhere. Multi-chunk levels store chunks as
[128, nch, 2, 512] slabs so every chunk's h-write is one contiguous
[128, 2, 512] block.

Chunks are 512 columns: matmuls run N=512 (one PSUM bank per MM).

Per internal chunk, one [128, 16, ncur] "GP" tile holds gates and
products interleaved so a single 6-block add produces [zs, zh, s]:
  blocks: zl(0:2) zr(2:4) Pz_l(4:6) Pz_r(6:8) Pr_l(8:10) Pr_r(10:12)
          rl(12:14) rr(14:16)
  PE : 32x (LDW + MM N=512) rzh  +  4x Wgh (back phase)
  ACT: 4 sigmoid (r-gates first), 1 tanh
  DVE: Pr = G[r]*H4, Pz = G[z]*H4, [zs,zh,s] = A + B (merged add),
       q = (zs-2)*g (fused STT), h = q*-0.5 + zh (fused STT)
  GPS: leaf h write only (zs on DVE).
Single-chunk levels get junk-MM warm-up bursts (HAM clock) and level-
gating backs split their tanh/q2/h chain per feature block so the next
level's left-child matmuls start earlier.

Host side only reshapes/permutes/casts numpy arrays and gathers.
"""

import os
from contextlib import ExitStack

import ml_dtypes
import numpy as np

import concourse.bass as bass
import concourse.mybir as mybir
import concourse.tile as tile
from concourse import bacc
from concourse.bass_utils import run_bass_kernel_spmd

F32 = mybir.dt.float32
BF16 = mybir.dt.bfloat16
MULT = mybir.AluOpType.mult
ADD = mybir.AluOpType.add
SUB = mybir.AluOpType.subtract
SIGMOID = mybir.ActivationFunctionType.Sigmoid
TANH = mybir.ActivationFunctionType.Tanh

MEM = 256
IN_DIM = 256
B = 64
L = 512
NCORES = 8
BLOC = B // NCORES            # trees per core
N0 = BLOC * L                 # leaf columns per core = 4096
NLEVELS = 10                  # 4096,2048,...,8 columns
NCOLS = [N0 >> l for l in range(NLEVELS)]
TOT = sum(NCOLS)              # 8184
OFFS = np.cumsum([0] + NCOLS).tolist()
NC = 512                      # node-column chunk

LAST_RESULT = {}


def _bitrev(n):
    """Bit-reversal permutation of range(n); perm[k] = rev(k). Involution."""
    bits = int(n).bit_length() - 1
    p = np.arange(n)
    r = np.zeros(n, dtype=np.int64)
    for _ in range(bits):
        r = (r << 1) | (p & 1)
        p >>= 1
    return r


def build_nc(fast_bias: bool):
    nc = bacc.Bacc("TRN2", target_bir_lowering=False, debug=False)

    d_x = nc.dram_tensor("xT", [2, 128, N0], BF16, kind="ExternalInput")
    d_wrzh = nc.dram_tensor("wrzh", [4, 128, 1024], BF16, kind="ExternalInput")
    d_wgrzx = nc.dram_tensor("wgrzx", [2, 128, 768], BF16, kind="ExternalInput")
    d_wgh = nc.dram_tensor("wgh", [2, 128, 256], BF16, kind="ExternalInput")
    d_bias = nc.dram_tensor("bias6", [6, 128, 1], F32, kind="ExternalInput")
    d_out = nc.dram_tensor("out", [2, 128, TOT], BF16, kind="ExternalOutput")

    x = d_x.ap()
    wrzh = d_wrzh.ap()
    wgrzx = d_wgrzx.ap()
    wgh = d_wgh.ap()
    bias6 = d_bias.ap()
    out = d_out.ap()

    mm = nc.tensor.matmul
    nchunks = [max(1, NCOLS[lv] // NC) for lv in range(NLEVELS)]

    with tile.TileContext(nc) as tc, ExitStack() as ctx:
        singles = ctx.enter_context(tc.tile_pool(name="singles", bufs=1))
        xpool = ctx.enter_context(tc.tile_pool(name="xpool", bufs=1))
        gpool = ctx.enter_context(tc.tile_pool(name="gates", bufs=3))
        szpool = ctx.enter_context(tc.tile_pool(name="szp", bufs=6))
        bpool = ctx.enter_context(tc.tile_pool(name="backp", bufs=6))
        psum = ctx.enter_context(tc.tile_pool(name="psum", bufs=1, space="PSUM"))

        # --- constants: spread initial DMAs across queues so x loads can
        # start immediately ---
        w_grzx = []
        for kc in range(2):
            t = singles.tile([128, 768], BF16, tag=f"wgrzx{kc}", name=f"wgrzx{kc}")
            nc.sync.dma_start(out=t, in_=wgrzx[kc])
            w_grzx.append(t)
        w_rzh = []
        for kc in range(4):
            t = singles.tile([128, 1024], BF16, tag=f"wrzh{kc}", name=f"wrzh{kc}")
            nc.scalar.dma_start(out=t, in_=wrzh[kc])
            w_rzh.append(t)
        w_gh = []
        for kc in range(2):
            t = singles.tile([128, 256], BF16, tag=f"wgh{kc}", name=f"wgh{kc}")
            nc.scalar.dma_start(out=t, in_=wgh[kc])
            w_gh.append(t)
        b_t = []
        if not fast_bias:
            for i in range(6):
                t = singles.tile([128, 1], F32, tag=f"b{i}", name=f"b{i}")
                nc.scalar.dma_start(out=t, in_=bias6[i])
                b_t.append(t)
        # b_t: [0]=bg0 [1]=bg1 [2]=bA0 [3]=bA1 [4]=bB0 [5]=bB1

        # h tiles: multi-chunk levels chunk-major [128, nch, 2, 512];
        # single-chunk levels [128, 2, n]
        h_t = []
        for lv in range(NLEVELS):
            if nchunks[lv] > 1:
                h_t.append(singles.tile([128, nchunks[lv], 2, NC], BF16,
                                        tag=f"h{lv}", name=f"h{lv}"))
            else:
                h_t.append(singles.tile([128, 2, NCOLS[lv]], BF16,
                                        tag=f"h{lv}", name=f"h{lv}"))

        def h4_view(lv, ci_child, ncur):
            """[128, 2(lr), 2(cb), ncur] view of level lv's h as children of
            the next level's chunk ci_child."""
            t = h_t[lv]
            nch = nchunks[lv]
            if nch > 1:
                return bass.AP(tensor=t.tensor, offset=t.offset + ci_child * 2 * NC,
                               ap=[list(t.ap[0]), [(nch // 2) * 2 * NC, 2],
                                   [NC, 2], [1, ncur]])
            n = NCOLS[lv]
            return bass.AP(tensor=t.tensor, offset=t.offset,
                           ap=[list(t.ap[0]), [n // 2, 2], [n, 2], [1, ncur]])

        def h_block(lv, kc, ci_child, ncur):
            """[128, ncur] matmul moving operand: contraction block kc of
            child_h = (lr = kc//2, cb = kc%2)."""
            t = h_t[lv]
            nch = nchunks[lv]
            if nch > 1:
                off = (t.offset + (kc // 2) * (nch // 2) * 2 * NC
                       + ci_child * 2 * NC + (kc % 2) * NC)
            else:
                n = NCOLS[lv]
                off = t.offset + (kc // 2) * (n // 2) + (kc % 2) * n
            return bass.AP(tensor=t.tensor, offset=off,
                           ap=[list(t.ap[0]), [1, ncur]])

        def h_dst(lv, ci, ncur):
            """Contiguous [128, 2, ncur] destination for chunk ci's h."""
            t = h_t[lv]
            if nchunks[lv] > 1:
                return t[:, ci, :, :]
            return t

        def out_dma(lv):
            t = h_t[lv]
            nch = nchunks[lv]
            for cb in range(2):
                if nch > 1:
                    src = bass.AP(tensor=t.tensor, offset=t.offset + cb * NC,
                                  ap=[list(t.ap[0]), [2 * NC, nch], [1, NC]])
                else:
                    src = t[:, cb, :]
                nc.sync.dma_start(out=out[cb, :, OFFS[lv]:OFFS[lv + 1]],
                                  in_=src)

        # H = 2h is stored everywhere: Wrzh/Wgh are pre-halved on the host
        # and the gather descales, so the back chain is 3 packed TTs:
        # u = 2 - zs ; v = u*g (= 2*tt*g) ; H = zh' + v   (zh' = 2zh via H)
        two = singles.tile([128, 2, NC], BF16, tag="two", name="two")
        nc.vector.memset(two, 2.0)

        state = {}   # (lv, ci) -> dict of tiles/views for the back phase

        # HAM warm-up: ~4us of junk matmuls as soon as the first weights
        # land, so the leaf stream starts at full clock
        warm = psum.tile([128, 2, NC], F32, tag="psg", name="warm", bufs=2)
        for _ in range(6):
            mm(warm[:, 0, :], w_grzx[0][:, 0:128], w_grzx[1][:, 0:NC],
               start=True, stop=True)

        x_groups = {}
        for g in range(N0 // NC):     # 8 groups of 512 cols, all resident
            t = xpool.tile([128, 2, NC], BF16, tag="x", name="xg", bufs=8)
            for kc in range(2):
                eng = nc.sync if (g + kc) % 2 == 0 else nc.gpsimd
                eng.dma_start(out=t[:, kc, :],
                              in_=x[kc, :, g * NC:(g + 1) * NC])
            x_groups[g] = t

        def emit_leaf_front(ci):
            c0 = ci * NC
            x_c = [x_groups[ci][:, kc, :] for kc in range(2)]
            # S4 = sigmoid(rzx) both halves: W out-blocks 2..5
            s4 = gpool.tile([128, 4, NC], BF16, tag="GP", name="s4")
            for q in range(2):
                ps = psum.tile([128, 2, NC], F32, tag="ps", name="ps_rzx",
                               bufs=2)
                for mb in range(2):
                    col = 256 + (q * 2 + mb) * 128
                    for kc in range(2):
                        mm(ps[:, mb, :], w_grzx[kc][:, col:col + 128],
                           x_c[kc], start=(kc == 0), stop=(kc == 1))
                if fast_bias:
                    nc.scalar.activation(s4[:, 2 * q:2 * q + 2, :], ps,
                                         SIGMOID, bias=1.0)
                else:
                    for mb in range(2):
                        nc.scalar.activation(
                            s4[:, 2 * q + mb, :], ps[:, mb, :],
                            SIGMOID, bias=b_t[2 + 2 * q + mb])
            ps_gx = psum.tile([128, 2, NC], F32, tag="psg", name="ps_gx",
                              bufs=2)
            for mb in range(2):
                for kc in range(2):
                    mm(ps_gx[:, mb, :], w_grzx[kc][:, 128 * mb:128 * mb + 128],
                       x_c[kc], start=(kc == 0), stop=(kc == 1))
            g2 = bpool.tile([128, 2, NC], BF16, tag="gsb", name="g2")
            if fast_bias:
                nc.scalar.activation(g2, ps_gx, TANH, bias=0.0)
            else:
                for mb in range(2):
                    nc.scalar.activation(g2[:, mb, :], ps_gx[:, mb, :],
                                         TANH, bias=b_t[mb])
            zs = szpool.tile([128, 2, NC], BF16, tag="lzs", name="zs", bufs=3)
            nc.vector.tensor_add(zs, s4[:, 0:2, :], s4[:, 2:4, :])
            u2 = bpool.tile([128, 2, NC], BF16, tag="q2", name="u2")
            nc.vector.tensor_sub(u2, two, zs)
            nc.vector.tensor_mul(h_dst(0, ci, NC), u2, g2)
            if ci == nchunks[0] - 1:
                out_dma(0)

        # GP block layout (in 2-block units):
        #   zl=0 zr=1 Pz_l=2 Pz_r=3 Pr_l=4 Pr_r=5 rl=6 rr=7
        # q order: r-gates first (rl, rr, zl, zr)
        QGATE = (6, 7, 0, 1)          # q -> GP 2-block slot
        QWBLK = (0, 2, 4, 6)          # q -> Wrzh out-block base (rl rr zl zr)

        def emit_front(lv, ci):
            if lv == 0:
                emit_leaf_front(ci)
                return
            n = NCOLS[lv]
            ncur = min(n, NC)
            big = nchunks[lv] > 1
            h4 = h4_view(lv - 1, ci, ncur)

            GP = gpool.tile([128, 16, ncur], BF16, tag="GP", name="GP")
            sz3 = szpool.tile([128, 6, ncur], BF16, tag="sz3", name="sz3")

            def rz_mms(q, pv):
                if q == 0 and lv >= 3:
                    # spine warm-up: junk MMs fill the PE-idle window while
                    # this level waits for h, keeping HAM at full clock
                    nj = 2900 // max(90, int(ncur // 2.2))
                    for _ in range(nj):
                        mm(pv[:, 0, :], w_rzh[0][:, 0:128],
                           w_rzh[1][:, 0:ncur], start=True, stop=True)
                for mb in range(2):
                    col = (QWBLK[q] + mb) * 128
                    for kc in range(4):
                        mm(pv[:, mb, :], w_rzh[kc][:, col:col + 128],
                           h_block(lv - 1, kc, ci, ncur),
                           start=(kc == 0), stop=(kc == 3))

            def act_q(dst2, psv, q):
                if fast_bias:
                    nc.scalar.activation(dst2, psv, SIGMOID, bias=1.0)
                else:
                    bi = (2, 4, 2, 4)[q]
                    for mb in range(2):
                        nc.scalar.activation(dst2[:, mb, :], psv[:, mb, :],
                                             SIGMOID, bias=b_t[bi + mb])

            mul_r = lambda: nc.vector.tensor_mul(
                GP[:, 8:12, :], GP[:, 12:16, :], h4)
            mul_z = lambda: nc.vector.tensor_mul(
                GP[:, 4:8, :], GP[:, 0:4, :], h4)

            if False:
                # whole rzh in one PSUM bank; one 8-block sigmoid with a
                # two-run dst (r-gates at blocks 12:16, z-gates at 0:4)
                ps = psum.tile([128, 8, ncur], F32, tag="ps", name="ps_rz",
                               bufs=2)
                for q in range(4):
                    rz_mms(q, ps[:, 2 * q:2 * q + 2, :])
                if fast_bias:
                    dst = bass.AP(
                        tensor=GP.tensor, offset=GP.offset + 12 * ncur,
                        ap=[list(GP.ap[0]), [-12 * ncur, 2], [1, 4 * ncur]])
                    nc.scalar.activation(dst, ps, SIGMOID, bias=1.0)
                else:
                    for q in range(4):
                        gs = QGATE[q]
                        act_q(GP[:, 2 * gs:2 * gs + 2, :],
                              ps[:, 2 * q:2 * q + 2, :], q)
                mul_r()
                mul_z()
            elif 4 * ncur <= 1024:
                # q-pairs share a 2-bank psum tile -> 4-block ACTs
                for pair in range(2):
                    ps = psum.tile([128, 4, ncur], F32, tag="ps",
                                   name="ps_rz", bufs=2)
                    for sub in range(2):
                        rz_mms(2 * pair + sub, ps[:, 2 * sub:2 * sub + 2, :])
                    if fast_bias:
                        gs0 = QGATE[2 * pair]
                        nc.scalar.activation(GP[:, 2 * gs0:2 * gs0 + 4, :],
                                             ps, SIGMOID, bias=1.0)
                    else:
                        for sub in range(2):
                            q = 2 * pair + sub
                            gs = QGATE[q]
                            act_q(GP[:, 2 * gs:2 * gs + 2, :],
                                  ps[:, 2 * sub:2 * sub + 2, :], q)
                    (mul_r if pair == 0 else mul_z)()
            else:
                # big chunks: per-q ACT, 2-bank psum tiles
                for q in range(4):
                    ps = psum.tile([128, 2, ncur], F32, tag="ps",
                                   name="ps_rz", bufs=2)
                    rz_mms(q, ps)
                    gs = QGATE[q]
                    act_q(GP[:, 2 * gs:2 * gs + 2, :], ps, q)
                    if q == 1:
                        mul_r()
                    elif q == 3:
                        mul_z()
            # [zs, zh, s] = [zl,Pz_l,Pr_l] + [zr,Pz_r,Pr_r] (stride-4 grps)
            A = bass.AP(tensor=GP.tensor, offset=GP.offset,
                        ap=[list(GP.ap[0]), [4 * ncur, 3], [ncur, 2],
                            [1, ncur]])
            Bv = bass.AP(tensor=GP.tensor, offset=GP.offset + 2 * ncur,
                         ap=[list(GP.ap[0]), [4 * ncur, 3], [ncur, 2],
                             [1, ncur]])
            nc.vector.tensor_add(sz3, A, Bv)
            state[(lv, ci)] = dict(sz3=sz3, c0=ci * ncur, ncur=ncur)

        def emit_back(lv, ci):
            if lv == 0:
                return
            st = state.pop((lv, ci))
            sz3 = st["sz3"]
            ncur = st["ncur"]
            psg = psum.tile([128, 2, ncur], F32, tag="psg", name="ps_g",
                            bufs=2)
            for mb in range(2):
                for kc in range(2):
                    mm(psg[:, mb, :], w_gh[kc][:, 128 * mb:128 * mb + 128],
                       sz3[:, 4 + kc, :], start=(kc == 0), stop=(kc == 1))
            g_sb = bpool.tile([128, 2, ncur], BF16, tag="gsb", name="g_sb")
            q2 = bpool.tile([128, 2, ncur], BF16, tag="q2", name="q2")
            v2 = bpool.tile([128, 2, ncur], BF16, tag="v2", name="v2")
            hd = h_dst(lv, ci, ncur)
            last = ci == nchunks[lv] - 1
            if last and ncur >= 256:
                # this back gates the next level: split the tanh/.../H chain
                # per feature-block so the first H half lands earlier
                for mb in range(2):
                    nc.scalar.activation(g_sb[:, mb, :], psg[:, mb, :], TANH,
                                         bias=0.0 if fast_bias else b_t[mb])
                    if ncur == NC:
                        nc.vector.tensor_sub(q2[:, mb, :], two[:, mb, :],
                                             sz3[:, mb, :])
                        nc.vector.tensor_mul(v2[:, mb, :], q2[:, mb, :],
                                             g_sb[:, mb, :])
                        nc.vector.tensor_add(hd[:, mb, :], v2[:, mb, :],
                                             sz3[:, 2 + mb, :])
                    else:
                        nc.vector.scalar_tensor_tensor(
                            q2[:, mb, :], sz3[:, mb, :], 2.0, g_sb[:, mb, :],
                            SUB, MULT)
                        nc.vector.scalar_tensor_tensor(
                            hd[:, mb, :], q2[:, mb, :], -1.0,
                            sz3[:, 2 + mb, :], MULT, ADD)
            else:
                if fast_bias:
                    nc.scalar.activation(g_sb, psg, TANH, bias=0.0)
                else:
                    for mb in range(2):
                        nc.scalar.activation(g_sb[:, mb, :], psg[:, mb, :],
                                             TANH, bias=b_t[mb])
                if ncur == NC:
                    nc.vector.tensor_sub(q2, two[:, :, 0:ncur], sz3[:, 0:2, :])
                    nc.vector.tensor_mul(v2, q2, g_sb)
                    nc.vector.tensor_add(hd, v2, sz3[:, 2:4, :])
                else:
                    nc.vector.scalar_tensor_tensor(q2, sz3[:, 0:2, :], 2.0,
                                                   g_sb, SUB, MULT)
                    nc.vector.scalar_tensor_tensor(
                        hd, q2, -1.0, sz3[:, 2:4, :], MULT, ADD)
            if last:
                out_dma(lv)

        # --- wavefront emission -------------------------------------------
        def parent_list(lv, ci):
            if lv == 0:
                return []
            np_ = nchunks[lv - 1]
            if np_ >= 2:
                return [(lv - 1, ci), (lv - 1, np_ // 2 + ci)]
            return [(lv - 1, 0)]

        def emit_junk(lv):
            # spine warm-up: junk MMs emitted BEFORE the parent backs so
            # they fill both the back's s-wait and this level's h-wait,
            # keeping HAM at full clock across the level boundary
            ncur = min(NCOLS[lv], NC)
            nj = 2900 // max(90, int(ncur // 2.2))
            jt = psum.tile([128, 2, ncur], F32, tag="psg", name="junk",
                           bufs=2)
            for _ in range(nj):
                mm(jt[:, 0, :], w_rzh[0][:, 0:128],
                   w_rzh[1][:, 0:ncur], start=True, stop=True)

        D = 2
        pending = []
        done = set()

        def pop_back():
            b = pending.pop(0)
            emit_back(*b)
            done.add(b)

        emitted = set()

        def emit_chunk(lv, ci):
            if (lv, ci) in emitted:
                return
            for par in parent_list(lv, ci):
                emit_chunk(*par)
            if lv >= 3 and nchunks[lv] == 1:
                emit_junk(lv)
            for par in parent_list(lv, ci):
                while par not in done:
                    pop_back()
            emitted.add((lv, ci))
            emit_front(lv, ci)
            if lv > 0:
                pending.append((lv, ci))
            else:
                done.add((lv, ci))
            while len(pending) > D:
                pop_back()

        for lv in range(NLEVELS):
            for ci in range(nchunks[lv]):
                emit_chunk(lv, ci)
        while pending:
            pop_back()

    nc.compile()
    return nc


def _prep_inputs(inputs, Wgrzx, bgrzx, Wrzh, Wgh):
    """Host-side shard + layout prep. Returns (in_maps, fast_bias)."""
    x = np.ascontiguousarray(inputs, dtype=np.float32)
    Wgrzx = np.asarray(Wgrzx, dtype=np.float32)
    bgrzx = np.asarray(bgrzx, dtype=np.float32)
    Wrzh = np.asarray(Wrzh, dtype=np.float32)
    Wgh = np.asarray(Wgh, dtype=np.float32)

    fast_bias = bool(
        np.all(bgrzx[:MEM] == 0.0) and np.all(bgrzx[MEM:] == 1.0))

    wgrzxT = np.ascontiguousarray(
        Wgrzx.T.reshape(2, 128, 768)).astype(ml_dtypes.bfloat16)
    wrzhT = np.ascontiguousarray(
        (Wrzh * 0.5).T.reshape(4, 128, 1024)).astype(ml_dtypes.bfloat16)
    wghT = np.ascontiguousarray(
        (Wgh * 0.5).T.reshape(2, 128, 256)).astype(ml_dtypes.bfloat16)
    bias6 = np.ascontiguousarray(bgrzx.reshape(6, 128, 1))

    br = _bitrev(N0)
    in_maps = []
    for c in range(NCORES):
        xc = x[c * BLOC:(c + 1) * BLOC].reshape(N0, IN_DIM)[br]
        xT = np.ascontiguousarray(xc.T).reshape(2, 128, N0).astype(
            ml_dtypes.bfloat16)
        in_maps.append({
            "xT": xT,
            "wrzh": wrzhT,
            "wgrzx": wgrzxT,
            "wgh": wghT,
            "bias6": bias6,
        })
    return in_maps, fast_bias


def _gather(results):
    """results: list of per-core {'out': [2,128,TOT] bf16} -> [B,2L-1,MEM]."""
    outs = []
    for c in range(len(results)):
        fm = np.asarray(results[c]["out"]).astype(np.float32).reshape(MEM, TOT) * 0.5
        levels = []
        for lv in range(NLEVELS):
            n = NCOLS[lv]
            blk = fm[:, OFFS[lv]:OFFS[lv + 1]]
            nat = blk[:, _bitrev(n)] if n > 1 else blk
            k = n // BLOC
            levels.append(nat.reshape(MEM, BLOC, k).transpose(1, 2, 0))
        outs.append(np.concatenate(levels, axis=1))
    return np.ascontiguousarray(
        np.concatenate(outs, axis=0), dtype=np.float32)


def kernel(**inputs):
    in_maps, fast_bias = _prep_inputs(
        inputs["inputs"], inputs["Wgrzx"], inputs["bgrzx"],
        inputs["Wrzh"], inputs["Wgh"])
    nc = build_nc(fast_bias)
    trace = bool(int(os.environ.get("BTGRU_TRACE", "0")))
    res = run_bass_kernel_spmd(
        nc, in_maps, core_ids=list(range(NCORES)), trace=trace)
    LAST_RESULT.clear()
    LAST_RESULT["exec_time_ns"] = res.exec_time_ns
    LAST_RESULT["profile_json"] = res.profile_json
    return _gather(res.results)
